# revision 26
# baseline (speedup 1.0000x reference)
"""Trainium2 Bass kernel for nn_CascadeEmbedding (embedding lookup + cascade fusion
+ 3-layer post-norm transformer encoder), distributed over 8 NeuronCores.

Sharding: 8 shards = (batch row b in 0..3) x (sequence half h in 0..1); each core
owns 256 tokens end-to-end. One pair-group AllGather per layer exchanges the
layer input; each core places its OWN 256 tokens at kv tiles 0-1 and reads the
partner half back with a rank-aware indirect DMA (per-core row-offset input),
so only the foreign half of K/V/scores waits on the collective. Activations are
feature-major [768, tokens]; weights host-pre-transposed so every matmul is
W_T.T @ X on the PE. Attention matmuls run bf16; the FFN runs fp8e4 with
DoubleRow (K=256 per instruction) and host-side weight scaling; the embedding
table is fp16. Weight streams use few large DMAs with double-buffered pools so
the next layer's slabs prefetch behind compute.
"""
import sys
sys.path.insert(0, '/opt/trn_rl_repo')
import numpy as np

B, S, V, NCC, EE, H, NH, HD, FF, NL = 4, 512, 50000, 1000, 256, 768, 12, 64, 3072, 3
NN = 13
T = 256                 # tokens per core
TK = 512                # row tokens (kv length)
HC = H // 128
FC = FF // 128
KVT = TK // 128
NCORES = 8
NG = 14                 # gather rounds per token group: 1 tok + 13 cascade
GSPLIT = [4, 4, 3, 3]   # gather buffer column counts
ZROW = V + NN * NCC
TROWS = ZROW + 1
import os as _os0
FP8_FFN = _os0.environ.get('KERNEL_FP8', '0') == '1'   # fp8 FFN misses the 2e-2 gate (3.8e-2)
WS1 = 64.0              # host-side fp8 weight scale for ff1
WS2 = 64.0              # host-side fp8 weight scale for ff2

_CACHE = {}


def _pin_act_table():
    """Constrain the ACT table-set picker to natural_log_exp_and_others (it
    contains every function this kernel uses: exp, ln, square, relu, copy),
    so exactly one table load is emitted instead of per-LN ping-pong."""
    import concourse.bacc as bacc_mod
    import concourse.hw_specs as hw
    if getattr(bacc_mod, '_act_tables_pinned', False):
        return
    orig = hw.get_activation_tables

    def patched(arch):
        t = orig(arch)
        return {k: (v if k == 'natural_log_exp_and_others' else set())
                for k, v in t.items()}
    bacc_mod.get_activation_tables = patched
    bacc_mod._act_tables_pinned = True


def _build_nc(reps=1, sim=False):
    import concourse.bass as bass
    import concourse.mybir as mybir
    import concourse.tile as tile
    from concourse import bacc
    _pin_act_table()

    F32R = mybir.dt.float32r
    F32 = mybir.dt.float32
    F16 = mybir.dt.float16
    BF16 = mybir.dt.bfloat16
    FP8 = mybir.dt.float8e4
    I32 = mybir.dt.int32
    AF = mybir.ActivationFunctionType
    OP = mybir.AluOpType
    AX = mybir.AxisListType
    DR = mybir.MatmulPerfMode.DoubleRow

    nc = bacc.Bacc(None, target_bir_lowering=False, num_swdge_queues=4,
                   num_devices=(1 if sim else NCORES))

    # ---------------- I/O ----------------
    table = nc.dram_tensor("table", [TROWS, H], F16, kind="ExternalInput")
    gids = nc.dram_tensor("gids", [128, 2 * NG], I32, kind="ExternalInput")
    posb = nc.dram_tensor("posb", [128, 2 * H], F16, kind="ExternalInput")
    cwx = nc.dram_tensor("cwx", [2 * NN, T], F32R, kind="ExternalInput")
    gcmat = nc.dram_tensor("gcmat", [2 * NN, H], F32R, kind="ExternalInput")
    ln0w = nc.dram_tensor("ln0w", [128, 2 * H], F32R, kind="ExternalInput")
    cid = nc.dram_tensor("cid", [128, 132], F32R, kind="ExternalInput")
    hotmat = nc.dram_tensor("hotmat", [128, NH * NH], BF16, kind="ExternalInput")
    selmat = nc.dram_tensor("selmat", [NH, H], F32R, kind="ExternalInput")
    onesall = nc.dram_tensor("onesall", [1, 256], F32R, kind="ExternalInput")
    agoff = nc.dram_tensor("agoff", [128, 1], I32, kind="ExternalInput")
    neg2 = nc.dram_tensor("neg2", [2, T], F32R, kind="ExternalInput")
    gbw = nc.dram_tensor("gbw", [NL * 2, 3, H], F32R, kind="ExternalInput")
    wq_m = nc.dram_tensor("wq_m", [NL, 128, 2, HC, 384], BF16, kind="ExternalInput")
    wk_m = nc.dram_tensor("wk_m", [NL, 128, 2, HC, 384], BF16, kind="ExternalInput")
    wo_m = nc.dram_tensor("wo_m", [NL, 128, 2, HC, 384], BF16, kind="ExternalInput")
    wv_m = nc.dram_tensor("wv_m", [NL, 128, 2 * HC, 384], BF16, kind="ExternalInput")
    if FP8_FFN:
        w1f = nc.dram_tensor("w1f", [NL, 128, FC, 3, 2, 128], FP8,
                             kind="ExternalInput")
        w2f = nc.dram_tensor("w2f", [NL, 128, FC // 2, HC, 2, 128], FP8,
                             kind="ExternalInput")
        f2br = nc.dram_tensor("f2br", [NL, 1, H], F32R, kind="ExternalInput")
    else:
        w1_s = nc.dram_tensor("w1_s", [NL, 8, 128, HC, 384], BF16,
                              kind="ExternalInput")
        w2_s = nc.dram_tensor("w2_s", [NL, FC, 128, H], BF16,
                              kind="ExternalInput")
    bvec = nc.dram_tensor("bvec", [NL, 128, 36], F32, kind="ExternalInput")
    y_out = nc.dram_tensor("y", [H, T], F32R, kind="ExternalOutput")

    import contextlib
    with tile.TileContext(nc) as tc, contextlib.ExitStack() as es:
        ec = es.enter_context
        ec(nc.allow_low_precision(reason="bf16/fp8 pipeline; stats stay fp32"))
        cpool = ec(tc.tile_pool(name="const", bufs=1))
        embp = ec(tc.tile_pool(name="emb", bufs=4))
        xsp = ec(tc.tile_pool(name="xstate", bufs=2))
        actp = ec(tc.tile_pool(name="act1", bufs=1))
        lntp = ec(tc.tile_pool(name="lnt", bufs=2))
        rowp = ec(tc.tile_pool(name="rows", bufs=1))
        lnwp = ec(tc.tile_pool(name="lnw", bufs=2))
        x1p = ec(tc.tile_pool(name="x1p", bufs=1))
        big1 = ec(tc.tile_pool(name="big1", bufs=1))
        wqkp = ec(tc.tile_pool(name="wqk", bufs=1))
        wvp = ec(tc.tile_pool(name="wvp", bufs=1))
        wop = ec(tc.tile_pool(name="wop", bufs=1))
        wfp = ec(tc.tile_pool(name="wfp", bufs=1))
        smp = ec(tc.tile_pool(name="small", bufs=4))
        psp = ec(tc.tile_pool(name="psum", bufs=8, space="PSUM"))
        dramp = ec(tc.tile_pool(name="dram", bufs=2, space="DRAM"))
        if True:
            def ps_tile(name):
                return psp.tile([128, 512], F32, tag="ps", name=name)

            # ------------- constants (gather indices first: gathers gate all) --
            gids_sb = cpool.tile([128, 2 * NG], I32)
            nc.sync.dma_start(gids_sb[:], gids[:])
            posb_sb = actp.tile([128, 2 * H], F16, tag="xq", name="posb_sb")
            nc.sync.dma_start(posb_sb[:], posb[:])
            cwx_sb = cpool.tile([2 * NN, T], F32R)
            nc.sync.dma_start(cwx_sb[:], cwx[:])
            gc_sb = cpool.tile([2 * NN, H], F32R)
            nc.sync.dma_start(gc_sb[:], gcmat[:])
            cid_sb = cpool.tile([128, 132], F32R)
            nc.sync.dma_start(cid_sb[:], cid[:])
            ident = cid_sb[:, 0:128]
            ones_col = cid_sb[:, 128:129]
            hot_sb = cpool.tile([128, NH * NH], BF16)
            nc.sync.dma_start(hot_sb[:], hotmat[:])
            selm_sb = cpool.tile([NH, H], F32R)
            nc.sync.dma_start(selm_sb[:], selmat[:])
            ones_sb = cpool.tile([1, 256], F32R)
            nc.sync.dma_start(ones_sb[:], onesall[:])
            agoff_sb = cpool.tile([128, 1], I32)
            nc.sync.dma_start(agoff_sb[:], agoff[:])
            rm_t = cpool.tile([2, T], F32R)
            nc.sync.dma_start(rm_t[:], neg2[:])
            ln0w_sb = actp.tile([128, 2 * H], F32R, tag="xln", name="ln0w_sb")
            nc.sync.dma_start(ln0w_sb[:], ln0w[:])
            eps0 = cpool.tile([128, 1], F32)
            nc.vector.memset(eps0[:], float(H) * H * 1e-12)
            epsl = cpool.tile([128, 1], F32)
            nc.vector.memset(epsl[:], float(H) * H * 1e-5)
            nhalf = cpool.tile([128, 1], F32)
            nc.vector.memset(nhalf[:], -0.5)
            if FP8_FFN:
                rs1 = cpool.tile([128, 1], F32)
                nc.vector.memset(rs1[:], 1.0 / WS1)

            lnargs = dict(nc=nc, mybir=mybir, ps_tile=ps_tile, lnwp=lnwp,
                          lntp=lntp, rowp=rowp, gbw=gbw, ones_col=ones_col,
                          rm_t=rm_t, epsl=epsl, nhalf=nhalf)

            # ------------- embedding + cascade + LN0 (token-major) -------------
            x0tok = []
            for t in range(2):
                # multi-row gathers: buffer i holds GSPLIT[i] gathered rows/token
                gb = []
                o0 = 0
                for i, k in enumerate(GSPLIT):
                    # reuse e_t score-tile slots (disjoint lifetime, same size)
                    gt = big1.tile([128, k * H], F16, tag=f"e_{i}", name=f"g{t}_{i}")
                    for j in range(k):
                        # HW SWDGE only honors one offset per partition per op
                        nc.gpsimd.indirect_dma_start(
                            out=gt[:, j * H:(j + 1) * H],
                            out_offset=None,
                            in_=table[:],
                            in_offset=bass.IndirectOffsetOnAxis(
                                ap=gids_sb[:, t * NG + o0 + j:
                                           t * NG + o0 + j + 1],
                                axis=0),
                        )
                    gb.append(gt)
                    o0 += k
                # tree-reduce the 14 segments + posb
                red = []
                for i, k in enumerate(GSPLIT):
                    gt = gb[i]
                    nc.vector.tensor_tensor(gt[:, 0:H], gt[:, 0:H], gt[:, H:2 * H],
                                            op=OP.add)
                    if k == 4:
                        nc.vector.tensor_tensor(gt[:, 2 * H:3 * H], gt[:, 2 * H:3 * H],
                                                gt[:, 3 * H:4 * H], op=OP.add)
                    nc.vector.tensor_tensor(gt[:, 0:H], gt[:, 0:H], gt[:, 2 * H:3 * H],
                                            op=OP.add)
                    red.append(gt[:, 0:H])
                nc.vector.tensor_tensor(red[0], red[0], red[1], op=OP.add)
                nc.vector.tensor_tensor(red[2], red[2], red[3], op=OP.add)
                nc.vector.tensor_tensor(red[2], red[2],
                                        posb_sb[:, t * H:(t + 1) * H], op=OP.add)
                casc_ps = ps_tile(f"casc{t}")
                casc_ps2 = ps_tile(f"casc2_{t}")
                nc.tensor.matmul(casc_ps[:, 0:512],
                                 lhsT=cwx_sb[:, t * 128:(t + 1) * 128],
                                 rhs=gc_sb[:, 0:512], start=True, stop=True)
                nc.tensor.matmul(casc_ps2[:, 0:256],
                                 lhsT=cwx_sb[:, t * 128:(t + 1) * 128],
                                 rhs=gc_sb[:, 512:768], start=True, stop=True)
                xg = embp.tile([128, H], F32R, tag="emb", name=f"xg{t}")
                nc.vector.tensor_tensor(xg[:], red[0], red[2], op=OP.add)
                nc.vector.tensor_tensor(xg[:, 0:512], xg[:, 0:512], casc_ps[:, 0:512],
                                        op=OP.add)
                nc.vector.tensor_tensor(xg[:, 512:768], xg[:, 512:768],
                                        casc_ps2[:, 0:256], op=OP.add)
                # LN0 (token-major): var*H^2 = H*sum(x^2) - sum(x)^2
                s1 = smp.tile([128, 1], F32, tag="s1")
                nc.vector.tensor_reduce(s1[:], xg[:], axis=AX.X, op=OP.add)
                scr = embp.tile([128, H], F32R, tag="emb", name=f"scr{t}")
                s2 = smp.tile([128, 1], F32, tag="s2")
                nc.scalar.activation(scr[:], xg[:], AF.Square, accum_out=s2[:])
                s1sq = smp.tile([128, 1], F32, tag="s1sq")
                nc.scalar.activation(s1sq[:], s1[:], AF.Square)
                t1 = smp.tile([128, 1], F32, tag="t1")
                nc.vector.scalar_tensor_tensor(t1[:], s2[:], float(H), s1sq[:],
                                               op0=OP.mult, op1=OP.subtract)
                # rstd via exp(-0.5*ln(.)): keeps ACT in one table set (ln+exp)
                lnv = smp.tile([128, 1], F32, tag="lnv")
                nc.scalar.activation(lnv[:], t1[:], AF.Ln, bias=eps0[:, 0:1])
                rr0 = smp.tile([128, 1], F32, tag="rr0")
                nc.scalar.activation(rr0[:], lnv[:], AF.Exp, scale=nhalf[:, 0:1])
                mean = smp.tile([128, 1], F32, tag="mean")
                nc.vector.tensor_scalar_mul(mean[:], s1[:], 1.0 / H)
                nc.vector.tensor_scalar(xg[:], xg[:], mean[:, 0:1], rr0[:, 0:1],
                                        op0=OP.subtract, op1=OP.mult)
                # ln0w rows: [g*H | b]
                nc.vector.tensor_tensor(xg[:], xg[:], ln0w_sb[:, 0:H], op=OP.mult)
                xt = embp.tile([128, H], F32R, tag="emb", name=f"x0tok{t}")
                nc.vector.tensor_tensor(xt[:], xg[:], ln0w_sb[:, H:2 * H], op=OP.add)
                x0tok.append(xt)

            # bridge: transpose to feature-major contiguous x [128, HC*T]
            x0_all = xsp.tile([128, HC, T], F32R, tag="x_all", name="x0_all")
            for c in range(HC):
                for t in range(2):
                    tp = ps_tile(f"br{c}_{t}")
                    nc.tensor.matmul(tp[:, 0:128],
                                     lhsT=x0tok[t][:, c * 128:(c + 1) * 128],
                                     rhs=ident[:], start=True, stop=True)
                    nc.vector.tensor_copy(x0_all[:, c, t * 128:(t + 1) * 128],
                                          tp[:, 0:128])
            xcur_all = x0_all
            xcur = [xcur_all[:, c, :] for c in range(HC)]

            # ------------- transformer layers -------------
            for l in [ll % NL for ll in range(NL * reps)]:
                # ---- weight slab DMAs (few, large; pools double-buffer) ----
                wq_sb = wqkp.tile([128, 2, HC, 384], BF16, tag="wq", name="wq_sb")
                nc.sync.dma_start(wq_sb[:], wq_m[l])
                wk_sb = wqkp.tile([128, 2, HC, 384], BF16, tag="wk", name="wk_sb")
                nc.sync.dma_start(wk_sb[:], wk_m[l])
                wv_sb = wvp.tile([128, 2 * HC, 384], BF16, tag="wv", name="wv_sb")
                nc.sync.dma_start(wv_sb[:], wv_m[l])
                wo_sb = wop.tile([128, 2, HC, 384], BF16, tag="wo", name="wo_sb")
                nc.sync.dma_start(wo_sb[:], wo_m[l])
                if FP8_FFN:
                    w1_sb = wfp.tile([128, FC, 3, 2, 128], FP8, tag="w1",
                                     name="w1_sb")
                    nc.sync.dma_start(w1_sb[:], w1f[l])
                    w2_sb = wfp.tile([128, FC // 2, HC, 2, 128], FP8, tag="w2",
                                     name="w2_sb")
                    nc.sync.dma_start(w2_sb[:], w2f[l])
                    f2b_sb = lnwp.tile([1, H], F32R, tag="f2b", name="f2b_sb")
                    nc.scalar.dma_start(f2b_sb[:], f2br[l])

                # ---- AllGather x within pairs (contiguous, single ops) ----
                ag_in = dramp.tile([128, HC * T], BF16, tag="ag_in")
                ag_out = dramp.tile([256, HC * T], BF16, tag="ag_out")
                xq_all = actp.tile([128, HC, T], BF16, tag="xq", name="xq_all")
                xq = [xq_all[:, c, :] for c in range(HC)]
                # GpSimd is idle during layers; casts off the DVE hot path
                nc.gpsimd.tensor_copy(xq_all[:], xcur_all[:])
                nc.scalar.dma_start(ag_in[:], xq_all[:])
                import os as _os
                if sim or _os.environ.get('KERNEL_NOCOLL'):
                    nc.sync.dma_start(ag_out[0:128, :], ag_in[:])
                    nc.sync.dma_start(ag_out[128:256, :], ag_in[:])
                else:
                    nc.gpsimd.collective_compute(
                        "AllGather", OP.bypass,
                        replica_groups=[[0, 1], [2, 3], [4, 5], [6, 7]],
                        ins=[ag_in[:].opt()], outs=[ag_out[:].opt()],
                    )

                bv_sb = smp.tile([128, 36], F32, tag="bv")
                nc.scalar.dma_start(bv_sb[:], bvec[l])

                # ---- OWN-half phase (no collective dependency) ----
                # Q (own tokens) and K own-half columns
                q_t, k_t = [], []
                for c in range(HC):
                    qt_ = actp.tile([128, T], BF16, tag=f"q_{c}", name=f"q_{c}")
                    q_t.append(qt_)
                    kt_ = big1.tile([128, TK], BF16, tag=f"k_{c}", name=f"k_{c}")
                    k_t.append(kt_)
                for ms in range(2):
                    for mo in range(3):
                        m = ms * 3 + mo
                        qp = ps_tile(f"qp{m}")
                        for k in range(HC):
                            nc.tensor.matmul(qp[:, 0:T],
                                             lhsT=wq_sb[:, ms, k,
                                                        mo * 128:(mo + 1) * 128],
                                             rhs=xq[k][:],
                                             start=(k == 0), stop=(k == HC - 1))
                        nc.scalar.copy(q_t[m][:], qp[:, 0:T])
                        kp = ps_tile(f"kp{m}")
                        for k in range(HC):
                            nc.tensor.matmul(kp[:, 0:T],
                                             lhsT=wk_sb[:, ms, k,
                                                        mo * 128:(mo + 1) * 128],
                                             rhs=xq[k][:],
                                             start=(k == 0), stop=(k == HC - 1))
                        nc.scalar.copy(k_t[m][:, 0:T], kp[:, 0:T])

                # V own tiles (kv tiles 0,1), token-major [kv, d] bf16
                v_tm = []
                for kt in range(KVT):
                    vt_ = actp.tile([128, H], BF16, tag=f"v_{kt}", name=f"v_{kt}")
                    v_tm.append(vt_)
                for half in range(2):
                    vps = [ps_tile(f"vp{half}_{kt}") for kt in range(2)]
                    for k in range(HC):
                        for kt in range(2):
                            nc.tensor.matmul(
                                vps[kt][:, 0:384],
                                lhsT=xq[k][:, kt * 128:(kt + 1) * 128],
                                rhs=wv_sb[:, half * HC + k, :],
                                start=(k == 0), stop=(k == HC - 1))
                    for kt in range(2):
                        nc.vector.tensor_copy(
                            v_tm[kt][:, half * 384:(half + 1) * 384],
                            vps[kt][:, 0:384])

                # scores + exp (head pairs share a psum bank; one exp per pair)
                # sums accumulate into 32-aligned psum rows: su bank A rows
                # {0,32,64,96} for m=0..3, bank B rows {0,32} for m=4,5
                e_t = []
                for kt in range(KVT):
                    et_ = big1.tile([128, NH * T], BF16, tag=f"e_{kt}", name=f"e_{kt}")
                    e_t.append(et_)
                su_ps = ps_tile("sums")
                nmm = [0]

                def pair_scores(m, kt):
                    # per-head psum banks: matmul outputs must start at a
                    # bank base (column-offset outputs hang the device)
                    for half in range(2):
                        hh = 2 * m + half
                        sp = ps_tile(f"sc{hh}_{kt}")
                        nc.tensor.matmul(
                            sp[:, 0:T],
                            lhsT=k_t[m][half * 64:half * 64 + 64,
                                        kt * 128:(kt + 1) * 128],
                            rhs=q_t[m][half * 64:half * 64 + 64, :],
                            start=True, stop=True)
                        nc.scalar.activation(
                            e_t[kt][:, hh * T:(hh + 1) * T], sp[:, 0:T], AF.Exp)
                    for half in range(2):
                        hh = 2 * m + half
                        nc.tensor.matmul(su_ps[0:NH, 0:T],
                                         lhsT=hot_sb[:, hh * NH:(hh + 1) * NH],
                                         rhs=e_t[kt][:, hh * T:(hh + 1) * T],
                                         start=(nmm[0] == 0), stop=(nmm[0] == 47))
                        nmm[0] += 1

                for m in range(HC):
                    for kt in range(2):
                        pair_scores(m, kt)

                # ---- FOREIGN-half phase: xf = (ag0 + ag1) - xq (exact) ----
                xf_all = actp.tile([128, HC, T], BF16, tag="xf", name="xf_all")
                xf = [xf_all[:, c, :] for c in range(HC)]
                # two pipelined halves: foreign-K matmuls start on the first
                # half while the second is still in flight
                for hb in range(2):
                    sl = slice(hb * 3 * T, (hb + 1) * 3 * T)
                    h0 = lntp.tile([128, 3 * T], BF16, tag=f"agh0_{hb}", bufs=1)
                    h1 = lntp.tile([128, 3 * T], BF16, tag=f"agh1_{hb}", bufs=1)
                    nc.scalar.dma_start(h0[:], ag_out[0:128, sl])
                    nc.sync.dma_start(h1[:], ag_out[128:256, sl])
                    nc.vector.tensor_tensor(xf_all[:, hb * 3:hb * 3 + 3, :],
                                            h0[:], h1[:], op=OP.add)
                    nc.vector.tensor_tensor(xf_all[:, hb * 3:hb * 3 + 3, :],
                                            xf_all[:, hb * 3:hb * 3 + 3, :],
                                            xq_all[:, hb * 3:hb * 3 + 3, :],
                                            op=OP.subtract)

                # K foreign-half columns
                for ms in range(2):
                    for mo in range(3):
                        m = ms * 3 + mo
                        kp = ps_tile(f"kf{m}")
                        for k in range(HC):
                            nc.tensor.matmul(kp[:, 0:T],
                                             lhsT=wk_sb[:, ms, k,
                                                        mo * 128:(mo + 1) * 128],
                                             rhs=xf[k][:],
                                             start=(k == 0), stop=(k == HC - 1))
                        nc.scalar.copy(k_t[m][:, T:TK], kp[:, 0:T])

                # V foreign tiles (kv tiles 2,3)
                for half in range(2):
                    vps = [ps_tile(f"vpf{half}_{kt}") for kt in range(2)]
                    for k in range(HC):
                        for kt in range(2):
                            nc.tensor.matmul(
                                vps[kt][:, 0:384],
                                lhsT=xf[k][:, kt * 128:(kt + 1) * 128],
                                rhs=wv_sb[:, half * HC + k, :],
                                start=(k == 0), stop=(k == HC - 1))
                    for kt in range(2):
                        nc.vector.tensor_copy(
                            v_tm[2 + kt][:, half * 384:(half + 1) * 384],
                            vps[kt][:, 0:384])

                # scores + exp + sums for foreign kv tiles
                for m in range(HC):
                    for kt in range(2, KVT):
                        pair_scores(m, kt)

                rec12 = rowp.tile([NH, T], F32R, tag="rec12")
                nc.vector.reciprocal(rec12[:], su_ps[0:NH, 0:T])

                # ---- attn = V^T @ E, normalized ----
                attn = []
                for m in range(HC):
                    ap_ = ps_tile(f"att{m}")
                    for half in range(2):
                        for kt in range(KVT):
                            hh = 2 * m + half
                            nc.tensor.matmul(
                                ap_[half * 64:half * 64 + 64, 0:T],
                                lhsT=v_tm[kt][:, hh * 64:(hh + 1) * 64],
                                rhs=e_t[kt][:, hh * T:(hh + 1) * T],
                                start=(kt == 0), stop=(kt == KVT - 1))
                    rb_ps = ps_tile(f"rb{m}")
                    nc.tensor.matmul(rb_ps[:, 0:T],
                                     lhsT=selm_sb[:, m * 128:(m + 1) * 128],
                                     rhs=rec12[:], start=True, stop=True)
                    rb = lntp.tile([128, T], F32, tag="rb")
                    nc.scalar.copy(rb[:], rb_ps[:, 0:T])
                    at = actp.tile([128, T], BF16, tag=f"attn_{m}", name=f"attn_{m}")
                    nc.vector.tensor_tensor(at[:], ap_[:, 0:T], rb[:], op=OP.mult)
                    attn.append(at)

                # ---- out-proj + bias + residual ----
                x1_all = x1p.tile([128, HC, T], F32R, tag="xt_all", name="x1_all")
                x1 = [x1_all[:, m, :] for m in range(HC)]
                for ms in range(2):
                    for mo in range(3):
                        m = ms * 3 + mo
                        op_ = ps_tile(f"op{m}")
                        for k in range(HC):
                            nc.tensor.matmul(op_[:, 0:T],
                                             lhsT=wo_sb[:, ms, k,
                                                        mo * 128:(mo + 1) * 128],
                                             rhs=attn[k][:],
                                             start=(k == 0), stop=(k == HC - 1))
                        nc.vector.scalar_tensor_tensor(
                            x1[m][:], op_[:, 0:T], bv_sb[:, m:m + 1], xcur[m][:],
                            op0=OP.add, op1=OP.add)

                # ---- LN1 ----
                xln_all, xln = _layer_norm(xin=x1, lni=l * 2, outpool=actp,
                                           outtag="xln", **lnargs)

                # ---- FFN ----
                if FP8_FFN:
                    # cast LN1 out to fp8 (contiguous [128, HC, T])
                    xq8 = actp.tile([128, HC, T], FP8, tag="xq8", name="xq8")
                    nc.gpsimd.tensor_copy(xq8[:], xln_all[:])
                    f2ps = [ps_tile(f"f2ps_{m}") for m in range(HC)]
                    f2ps = [t[:, 0:T] for t in f2ps]
                    # ff2 bias row (scaled by WS2 host-side), rank-1 matmul
                    for m in range(HC):
                        nc.tensor.matmul(f2ps[m][:],
                                         lhsT=f2b_sb[0:1, m * 128:(m + 1) * 128],
                                         rhs=ones_sb[0:1, 0:T],
                                         start=True, stop=False,
                                         skip_group_check=True)
                    fm_q = {}
                    for kp2 in range(FC // 2):
                        fmp = lntp.tile([128, 2, T], FP8, tag="ffm",
                                        name=f"ffm_{kp2}")
                        for j in range(2):
                            fo = 2 * kp2 + j
                            fp_ = ps_tile(f"fp{fo}")
                            for kp in range(3):
                                nc.tensor.matmul(
                                    fp_[:, 0:T],
                                    lhsT=w1_sb[:, fo, kp, :, :],
                                    rhs=xq8[:, 2 * kp:2 * kp + 2, :],
                                    start=(kp == 0), stop=(kp == 2),
                                    perf_mode=DR)
                            nc.scalar.activation(fmp[:, j, :], fp_[:, 0:T],
                                                 AF.Relu,
                                                 bias=bv_sb[:, 6 + fo:7 + fo],
                                                 scale=rs1[:, 0:1])
                        fm_q[kp2] = fmp
                        for m in range(HC):
                            nc.tensor.matmul(
                                f2ps[m][:],
                                lhsT=w2_sb[:, kp2, m, :, :],
                                rhs=fmp[:, 0:2, :],
                                start=False, stop=(kp2 == FC // 2 - 1),
                                perf_mode=DR, skip_group_check=True)
                    x2_all = xsp.tile([128, HC, T], F32R, tag="x_all",
                                      name="x2_all")
                    x2 = [x2_all[:, m, :] for m in range(HC)]
                    for m in range(HC):
                        nc.vector.scalar_tensor_tensor(
                            x2[m][:], f2ps[m][:], 1.0 / WS2, xln[m][:],
                            op0=OP.mult, op1=OP.add)
                else:
                    xlnb = []
                    for c in range(HC):
                        xb_ = actp.tile([128, T], BF16, tag=f"xlnb_{c}",
                                        name=f"xlnb_{c}")
                        nc.gpsimd.tensor_copy(xb_[:], xln[c][:])
                        xlnb.append(xb_)
                    f2ps = [ps_tile(f"f2ps_{m}") for m in range(HC)]
                    f2ps = [t[:, 0:T] for t in f2ps]
                    # ff1 runs one mid-chunk ahead of ff2 so relu never stalls PE
                    fm_q = {}
                    wsl_q = {}

                    def emit_f2(fo):
                        for m in range(HC):
                            nc.tensor.matmul(f2ps[m][:],
                                             lhsT=wsl_q[fo][:, m * 128:(m + 1) * 128],
                                             rhs=fm_q[fo][:],
                                             start=(fo == 0), stop=(fo == FC - 1))
                    for sl in range(8):
                        fsl = wfp.tile([128, HC, 384], BF16, tag="w1slab", bufs=4)
                        nc.sync.dma_start(fsl[:], w1_s[l, sl])
                        for mo in range(3):
                            fo = sl * 3 + mo
                            fp = ps_tile(f"fp{fo}")
                            for k in range(HC):
                                nc.tensor.matmul(fp[:, 0:T],
                                                 lhsT=fsl[:, k,
                                                          mo * 128:(mo + 1) * 128],
                                                 rhs=xlnb[k][:],
                                                 start=(k == 0), stop=(k == HC - 1))
                            fm = lntp.tile([128, T], BF16, tag="ffm",
                                           name=f"ffm_{fo}")
                            nc.scalar.activation(fm[:], fp[:, 0:T], AF.Relu,
                                                 bias=bv_sb[:, 6 + fo:7 + fo])
                            fm_q[fo] = fm
                            wsl = wfp.tile([128, H], BF16, tag="w2slab", bufs=8)
                            nc.sync.dma_start(wsl[:], w2_s[l, fo])
                            wsl_q[fo] = wsl
                            if fo >= 1:
                                emit_f2(fo - 1)
                    emit_f2(FC - 1)
                    x2_all = xsp.tile([128, HC, T], F32R, tag="x_all",
                                      name="x2_all")
                    x2 = [x2_all[:, m, :] for m in range(HC)]
                    for m in range(HC):
                        nc.vector.scalar_tensor_tensor(
                            x2[m][:], f2ps[m][:], bv_sb[:, 30 + m:31 + m],
                            xln[m][:], op0=OP.add, op1=OP.add)

                # ---- LN2 -> next x ----
                xcur_all, xcur = _layer_norm(xin=x2, lni=l * 2 + 1, outpool=xsp,
                                             outtag="x_all", **lnargs)

            # ------------- output -------------
            for c in range(HC):
                nc.scalar.dma_start(y_out[c * 128:(c + 1) * 128, :], xcur[c][:])

    nc.compile()
    return nc


def _layer_norm(nc, mybir, ps_tile, lnwp, lntp, rowp, gbw, ones_col, rm_t, epsl,
                nhalf, xin, lni, outpool, outtag):
    """Feature-major layernorm over 6 chunks [128, T], writing a contiguous
    [128, 6, T] output tile (views returned).
    gbw rows: [g*H, g, b]; rr0 = 1/(H*std); a = gH (x) rr0; b_ps = g (x) rr0*S - b.
    """
    F32 = mybir.dt.float32
    F32R = mybir.dt.float32r
    AF = mybir.ActivationFunctionType
    OP = mybir.AluOpType
    H = 768
    gb = lnwp.tile([2, H], F32R, tag="gb", name=f"gb{lni}")
    nc.scalar.dma_start(gb[:], gbw[lni, 0:2])
    gh = lnwp.tile([1, H], F32R, tag="gh", name=f"gh{lni}")
    nc.scalar.dma_start(gh[:], gbw[lni, 2:3])
    s_ps = ps_tile(f"lns{lni}")
    q_ps = ps_tile(f"lnq{lni}")
    for c in range(6):
        sq = lntp.tile([128, 256], F32R, tag="lnsq")
        nc.scalar.activation(sq[:], xin[c][:], AF.Square)
        nc.tensor.matmul(s_ps[0:1, 0:256], lhsT=ones_col[:], rhs=xin[c][:],
                         start=(c == 0), stop=(c == 5))
        nc.tensor.matmul(q_ps[0:1, 0:256], lhsT=ones_col[:], rhs=sq[:],
                         start=(c == 0), stop=(c == 5))
    s2 = rowp.tile([1, 256], F32, tag="ls2")
    nc.scalar.activation(s2[:], s_ps[0:1, 0:256], AF.Square)
    t1 = rowp.tile([1, 256], F32, tag="lt1")
    nc.vector.scalar_tensor_tensor(t1[:], q_ps[0:1, 0:256], float(H), s2[:],
                                   op0=OP.mult, op1=OP.subtract)
    lnv = rowp.tile([1, 256], F32, tag="llnv")
    nc.scalar.activation(lnv[:], t1[:], AF.Ln, bias=epsl[0:1, 0:1])
    rr = rowp.tile([1, 256], F32R, tag="lr")
    nc.scalar.activation(rr[:], lnv[:], AF.Exp, scale=nhalf[0:1, 0:1])
    nc.vector.tensor_tensor(rm_t[0:1, :], rr[:], s_ps[0:1, 0:256], op=OP.mult)
    out_all = outpool.tile([128, 6, 256], F32R, tag=outtag,
                           name=f"{outtag}{lni}")
    out = []
    for c in range(6):
        a_ps = ps_tile(f"lna{lni}_{c}")
        nc.tensor.matmul(a_ps[:, 0:256], lhsT=gh[0:1, c * 128:(c + 1) * 128],
                         rhs=rr[:], start=True, stop=True)
        b_ps = ps_tile(f"lnb{lni}_{c}")
        nc.tensor.matmul(b_ps[:, 0:256], lhsT=gb[:, c * 128:(c + 1) * 128],
                         rhs=rm_t[:], start=True, stop=True)
        tt = lntp.tile([128, 256], F32R, tag="lnt")
        nc.vector.tensor_tensor(tt[:], xin[c][:], a_ps[:, 0:256], op=OP.mult)
        nc.vector.tensor_tensor(out_all[:, c, :], tt[:], b_ps[:, 0:256],
                                op=OP.subtract)
        out.append(out_all[:, c, :])
    return out_all, out


def _host_pack(inputs):
    import ml_dtypes
    f32 = np.float32
    f16 = np.float16
    bf = ml_dtypes.bfloat16
    f8 = ml_dtypes.float8_e4m3
    tok = np.asarray(inputs['tok_emb'], f32)
    pos = np.asarray(inputs['pos_emb'], f32)
    node = np.asarray(inputs['node_emb'], f32)
    cw_W = np.asarray(inputs['cw_W'], f32)
    cw_b = np.asarray(inputs['cw_b'], f32)
    fus_W = np.asarray(inputs['fus_W'], f32)
    fus_b = np.asarray(inputs['fus_b'], f32)
    ln_g = np.asarray(inputs['ln_g'], f32)
    ln_b = np.asarray(inputs['ln_b'], f32)
    iW = np.asarray(inputs['attn_in_W'], f32)
    ib = np.asarray(inputs['attn_in_b'], f32)
    oW = np.asarray(inputs['attn_out_W'], f32)
    ob = np.asarray(inputs['attn_out_b'], f32)
    f1W = np.asarray(inputs['ff1_W'], f32)
    f1b = np.asarray(inputs['ff1_b'], f32)
    f2W = np.asarray(inputs['ff2_W'], f32)
    f2b = np.asarray(inputs['ff2_b'], f32)
    g1 = np.asarray(inputs['ln1_g'], f32)
    b1 = np.asarray(inputs['ln1_b'], f32)
    g2 = np.asarray(inputs['ln2_g'], f32)
    b2 = np.asarray(inputs['ln2_b'], f32)
    input_ids = np.asarray(inputs['input_ids']).astype(np.int64)
    ccids = np.asarray(inputs['cascade_concept_ids']).astype(np.int64)
    cwts = np.asarray(inputs['cascade_weights'], f32)
    cmask = np.asarray(inputs['cascade_mask']).astype(bool)

    fw3 = fus_W.reshape(H, NN, EE)
    table = np.empty((TROWS, H), f16)
    table[:V] = tok.astype(f16)
    tn = np.matmul(node[None, :, :], fw3.transpose(1, 2, 0))
    table[V:V + NN * NCC] = tn.reshape(NN * NCC, H).astype(f16)
    table[ZROW] = 0.0
    G = np.einsum('e,hne->nh', cw_W[:, 0], fw3)
    C = np.einsum('e,hne->nh', cw_b, fw3)
    gcmat = np.concatenate([G, C], axis=0).astype(f32)

    cid = np.zeros((128, 132), f32)
    cid[:, :128] = np.eye(128, dtype=f32)
    cid[:, 128] = 1.0
    hotm = np.zeros((128, NH * NH), bf)
    for hh_ in range(NH):
        hotm[:, hh_ * NH + hh_] = 1.0
    selm = np.zeros((NH, H), f32)
    for m_ in range(HC):
        selm[2 * m_, m_ * 128:m_ * 128 + 64] = 1.0
        selm[2 * m_ + 1, m_ * 128 + 64:(m_ + 1) * 128] = 1.0
    onesall = np.ones((1, 256), f32)
    neg2 = np.full((2, T), -1.0, f32)
    ln0w = np.empty((128, 2 * H), f32)
    ln0w[:, :H] = np.broadcast_to(ln_g[None, :] * H, (128, H))
    ln0w[:, H:] = np.broadcast_to(ln_b[None, :], (128, H))
    gbw = np.empty((NL * 2, 3, H), f32)
    for l in range(NL):
        gbw[2 * l, 0], gbw[2 * l, 1], gbw[2 * l, 2] = g1[l], b1[l], g1[l] * H
        gbw[2 * l + 1, 0], gbw[2 * l + 1, 1], gbw[2 * l + 1, 2] = \
            g2[l], b2[l], g2[l] * H

    def mslab(wt, nslab):
        K, M = wt.shape
        w = M // nslab
        a = wt.reshape(K // 128, 128, M).transpose(1, 0, 2)
        return np.stack([a[:, :, i * w:(i + 1) * w] for i in range(nslab)], 0)

    wq_m = np.empty((NL, 128, 2, HC, 384), bf)
    wk_m = np.empty((NL, 128, 2, HC, 384), bf)
    wo_m = np.empty((NL, 128, 2, HC, 384), bf)
    wv_m = np.empty((NL, 128, 2 * HC, 384), bf)
    bvec = np.empty((NL, 128, 36), f32)
    if FP8_FFN:
        w1f = np.empty((NL, 128, FC, 3, 2, 128), f8)
        w2f = np.empty((NL, 128, FC // 2, HC, 2, 128), f8)
        f2br = np.empty((NL, 1, H), f32)
    else:
        w1_s = np.empty((NL, 8, 128, HC, 384), bf)
        w2_s = np.empty((NL, FC, 128, H), bf)
    for l in range(NL):
        wq_t = iW[l, 0:H, :].T * (1.0 / np.sqrt(HD))
        wk_t = iW[l, H:2 * H, :].T
        wv_t = iW[l, 2 * H:3 * H, :].T
        wq_m[l] = mslab(wq_t, 2).transpose(1, 0, 2, 3)
        wk_m[l] = mslab(wk_t, 2).transpose(1, 0, 2, 3)
        wo_m[l] = mslab(oW[l].T, 2).transpose(1, 0, 2, 3)
        for half in range(2):
            wv_m[l, :, half * HC:(half + 1) * HC] = \
                wv_t[:, half * 384:(half + 1) * 384].reshape(
                    HC, 128, 384).transpose(1, 0, 2)
        if FP8_FFN:
            # w1f[p, fo, kp, j, o] = f1W[fo*128+o, (2kp+j)*128+p] * WS1
            a = (f1W[l] * WS1).reshape(FC, 128, 6, 128)   # [fo, o, s, p]
            a = a.reshape(FC, 128, 3, 2, 128)             # [fo, o, kp, j, p]
            w1f[l] = a.transpose(4, 0, 2, 3, 1).astype(f8)
            # w2f[p, kp, m, j, o] = f2W[m*128+o, (2kp+j)*128+p] * WS2
            b = (f2W[l] * WS2).reshape(HC, 128, FC // 2, 2, 128)
            w2f[l] = b.transpose(4, 2, 0, 3, 1).astype(f8)
            f2br[l, 0] = f2b[l] * WS2
        else:
            w1_s[l] = mslab(f1W[l].T, 8)
            w2_s[l] = f2W[l].T.reshape(FC, 128, H).astype(bf)
        ob2 = ob[l] + oW[l] @ ib[l, 2 * H:3 * H]
        bvec[l, :, 0:6] = ob2.reshape(HC, 128).T
        bvec[l, :, 6:30] = f1b[l].reshape(FC, 128).T
        bvec[l, :, 30:36] = f2b[l].reshape(HC, 128).T

    shared = dict(table=table, gcmat=gcmat, cid=cid, hotmat=hotm,
                  selmat=selm, onesall=onesall, neg2=neg2, ln0w=ln0w, gbw=gbw,
                  wq_m=wq_m, wk_m=wk_m, wo_m=wo_m, wv_m=wv_m, bvec=bvec)
    if FP8_FFN:
        shared.update(w1f=w1f, w2f=w2f, f2br=f2br)
    else:
        shared.update(w1_s=w1_s, w2_s=w2_s)

    cwm = (cwts * cmask).astype(f32)
    in_maps = []
    for r in range(NCORES):
        b, hh = r // 2, r % 2
        ssl = slice(hh * T, (hh + 1) * T)
        sidx = np.arange(S)[ssl]
        gid = np.empty((128, 2 * NG), np.int32)
        pb = np.empty((128, 2 * H), f16)
        for t in range(2):
            rows = sidx[t * 128:(t + 1) * 128]
            gid[:, t * NG + 0] = input_ids[b, rows]
            for n in range(NN):
                cc = V + n * NCC + ccids[rows, n]
                cc = np.where(cmask[rows, n], cc, ZROW)
                gid[:, t * NG + 1 + n] = cc
            pb[:, t * H:(t + 1) * H] = (pos[rows] + fus_b[None, :]).astype(f16)
        cwxv = np.concatenate([cwm[ssl].T, cmask[ssl].T.astype(f32)], 0)
        m = dict(shared)
        m['gids'] = gid
        m['posb'] = pb
        m['cwx'] = np.ascontiguousarray(cwxv)
        # foreign-half row offsets into ag_out: partner block +(own partition)
        m['agoff'] = ((1 - hh) * 128 + np.arange(128, dtype=np.int32)
                      ).reshape(128, 1)
        in_maps.append(m)
    return in_maps


def _make_runner(reps=1):
    """Build nc once and return fn(in_maps) -> list of per-core result dicts,
    with the jitted executable cached for repeat timing."""
    import jax
    from jax.sharding import Mesh, PartitionSpec
    from jax.experimental.shard_map import shard_map
    import concourse.mybir as mybir
    from concourse import bass2jax
    from concourse.bass2jax import _bass_exec_p, install_neuronx_cc_hook, \
        partition_id_tensor

    nc = _build_nc(reps)
    install_neuronx_cc_hook()
    partition_name = nc.partition_id_tensor.name if nc.partition_id_tensor else None
    in_names, out_names, out_avals, zero_outs = [], [], [], []
    for alloc in nc.m.functions[0].allocations:
        if not isinstance(alloc, mybir.MemoryLocationSet):
            continue
        name = alloc.memorylocations[0].name
        if alloc.kind == "ExternalInput":
            if name != partition_name:
                in_names.append(name)
        elif alloc.kind == "ExternalOutput":
            out_names.append(name)
            shape = tuple(alloc.tensor_shape)
            dtype = mybir.dt.np(alloc.dtype)
            out_avals.append(jax.core.ShapedArray(shape, dtype))
            zero_outs.append(np.zeros(shape, dtype))
    n_params = len(in_names)
    n_outs = len(out_avals)
    all_names = in_names + out_names + ([partition_name] if partition_name else [])
    donate = tuple(range(n_params, n_params + n_outs))

    def _body(*args):
        operands = list(args)
        if partition_name is not None:
            operands.append(partition_id_tensor())
        outs = _bass_exec_p.bind(
            *operands,
            out_avals=tuple(out_avals),
            in_names=tuple(all_names),
            out_names=tuple(out_names),
            lowering_input_output_aliases=(),
            sim_require_finite=True,
            sim_require_nnan=True,
            nc=nc,
        )
        return tuple(outs)

    devices = jax.devices()[:NCORES]
    mesh = Mesh(np.asarray(devices), ("core",))
    in_specs = (PartitionSpec("core"),) * (n_params + n_outs)
    out_specs = (PartitionSpec("core"),) * len(out_names)
    sharded = jax.jit(
        shard_map(_body, mesh=mesh, in_specs=in_specs, out_specs=out_specs,
                  check_rep=False),
        donate_argnums=donate, keep_unused=True)

    def runner(in_maps, n_iters=1, dev_inputs=None):
        import time as _time
        if dev_inputs is None:
            concat_in = [np.concatenate([np.asarray(in_maps[c][nm])
                                         for c in range(NCORES)], axis=0)
                         for nm in in_names]
            dev_inputs = [jax.device_put(a) for a in concat_in]
        times = []
        out_arrs = None
        for _ in range(n_iters):
            concat_zeros = [np.zeros((NCORES * z.shape[0], *z.shape[1:]), z.dtype)
                            for z in zero_outs]
            t0 = _time.time()
            out_arrs = sharded(*dev_inputs, *concat_zeros)
            jax.block_until_ready(out_arrs)
            times.append(_time.time() - t0)
        results = [
            {nm: np.asarray(out_arrs[i]).reshape(NCORES, *out_avals[i].shape)[c]
             for i, nm in enumerate(out_names)}
            for c in range(NCORES)
        ]
        return results, times, dev_inputs

    return runner


def _run(inputs, trace=False, n_iters=1, reps=1):
    key = f'runner{reps}'
    if key not in _CACHE:
        _CACHE[key] = _make_runner(reps)
    runner = _CACHE[key]
    in_maps = _host_pack(inputs)
    results, times, _ = runner(in_maps, n_iters=n_iters)
    out = np.empty((B, S, H), np.float32)
    for r in range(NCORES):
        b, hh = r // 2, r % 2
        y = np.asarray(results[r]['y'])
        out[b, hh * T:(hh + 1) * T, :] = y.T

    class Res:
        pass
    res = Res()
    res.times = times
    res.exec_time_ns = None
    return out, res


def kernel(**inputs):
    out, _ = _run(inputs)
    return out


# revision 27
# speedup vs baseline: 1.1671x; 1.1671x over previous
"""Trainium2 Bass kernel for nn_CascadeEmbedding (embedding lookup + cascade fusion
+ 3-layer post-norm transformer encoder), distributed over 8 NeuronCores.

Sharding: 8 shards = (batch row b in 0..3) x (sequence half h in 0..1); each core
owns 256 tokens end-to-end. One pair-group AllGather per layer exchanges the
layer input; each core places its OWN 256 tokens at kv tiles 0-1 and reads the
partner half back with a rank-aware indirect DMA (per-core row-offset input),
so only the foreign half of K/V/scores waits on the collective. Activations are
feature-major [768, tokens]; weights host-pre-transposed so every matmul is
W_T.T @ X on the PE. Attention matmuls run bf16; the FFN runs fp8e4 with
DoubleRow (K=256 per instruction) and host-side weight scaling; the embedding
table is fp16. Weight streams use few large DMAs with double-buffered pools so
the next layer's slabs prefetch behind compute.
"""
import sys
sys.path.insert(0, '/opt/trn_rl_repo')
import numpy as np

B, S, V, NCC, EE, H, NH, HD, FF, NL = 4, 512, 50000, 1000, 256, 768, 12, 64, 3072, 3
NN = 13
T = 256                 # tokens per core
TK = 512                # row tokens (kv length)
HC = H // 128
FC = FF // 128
KVT = TK // 128
NCORES = 8
NG = 14                 # gather rounds per token group: 1 tok + 13 cascade
GSPLIT = [4, 4, 3, 3]   # gather buffer column counts
ZROW = V + NN * NCC
TROWS = ZROW + 1
import os as _os0
FP8_FFN = _os0.environ.get('KERNEL_FP8', '0') == '1'   # fp8 FFN misses the 2e-2 gate (3.8e-2)
WS1 = 64.0              # host-side fp8 weight scale for ff1
WS2 = 64.0              # host-side fp8 weight scale for ff2

_CACHE = {}


def _pin_act_table():
    """Constrain the ACT table-set picker to natural_log_exp_and_others (it
    contains every function this kernel uses: exp, ln, square, relu, copy),
    so exactly one table load is emitted instead of per-LN ping-pong."""
    import concourse.bacc as bacc_mod
    import concourse.hw_specs as hw
    if getattr(bacc_mod, '_act_tables_pinned', False):
        return
    orig = hw.get_activation_tables

    def patched(arch):
        t = orig(arch)
        return {k: (v if k == 'natural_log_exp_and_others' else set())
                for k, v in t.items()}
    bacc_mod.get_activation_tables = patched
    bacc_mod._act_tables_pinned = True


def _build_nc(reps=1, sim=False):
    import concourse.bass as bass
    import concourse.mybir as mybir
    import concourse.tile as tile
    from concourse import bacc
    _pin_act_table()

    F32R = mybir.dt.float32r
    F32 = mybir.dt.float32
    F16 = mybir.dt.float16
    BF16 = mybir.dt.bfloat16
    FP8 = mybir.dt.float8e4
    I32 = mybir.dt.int32
    AF = mybir.ActivationFunctionType
    OP = mybir.AluOpType
    AX = mybir.AxisListType
    DR = mybir.MatmulPerfMode.DoubleRow

    nc = bacc.Bacc(None, target_bir_lowering=False, num_swdge_queues=4,
                   num_devices=(1 if sim else NCORES))

    # ---------------- I/O ----------------
    table = nc.dram_tensor("table", [TROWS, H], F16, kind="ExternalInput")
    gids = nc.dram_tensor("gids", [128, 2 * NG], I32, kind="ExternalInput")
    posb = nc.dram_tensor("posb", [128, 2 * H], F16, kind="ExternalInput")
    cwx = nc.dram_tensor("cwx", [2 * NN, T], F32R, kind="ExternalInput")
    gcmat = nc.dram_tensor("gcmat", [2 * NN, H], F32R, kind="ExternalInput")
    ln0w = nc.dram_tensor("ln0w", [128, 2 * H], F32R, kind="ExternalInput")
    cid = nc.dram_tensor("cid", [128, 132], F32R, kind="ExternalInput")
    hotmat = nc.dram_tensor("hotmat", [128, NH * NH], BF16, kind="ExternalInput")
    selmat = nc.dram_tensor("selmat", [NH, H], F32R, kind="ExternalInput")
    onesall = nc.dram_tensor("onesall", [1, 256], F32R, kind="ExternalInput")
    agoff = nc.dram_tensor("agoff", [128, 1], I32, kind="ExternalInput")
    neg2 = nc.dram_tensor("neg2", [2, T], F32R, kind="ExternalInput")
    gbw = nc.dram_tensor("gbw", [NL * 2, 3, H], F32R, kind="ExternalInput")
    wq_m = nc.dram_tensor("wq_m", [NL, 128, 2, HC, 384], BF16, kind="ExternalInput")
    wk_m = nc.dram_tensor("wk_m", [NL, 128, 2, HC, 384], BF16, kind="ExternalInput")
    wo_m = nc.dram_tensor("wo_m", [NL, 128, 2, HC, 384], BF16, kind="ExternalInput")
    wv_m = nc.dram_tensor("wv_m", [NL, 128, 2 * HC, 384], BF16, kind="ExternalInput")
    if FP8_FFN:
        w1f = nc.dram_tensor("w1f", [NL, 128, FC, 3, 2, 128], FP8,
                             kind="ExternalInput")
        w2f = nc.dram_tensor("w2f", [NL, 128, FC // 2, HC, 2, 128], FP8,
                             kind="ExternalInput")
        f2br = nc.dram_tensor("f2br", [NL, 1, H], F32R, kind="ExternalInput")
    else:
        w1_s = nc.dram_tensor("w1_s", [NL, 8, 128, HC, 384], BF16,
                              kind="ExternalInput")
        w2_s = nc.dram_tensor("w2_s", [NL, FC, 128, H], BF16,
                              kind="ExternalInput")
    bvec = nc.dram_tensor("bvec", [NL, 128, 36], F32, kind="ExternalInput")
    y_out = nc.dram_tensor("y", [H, T], F32R, kind="ExternalOutput")

    import contextlib
    with tile.TileContext(nc) as tc, contextlib.ExitStack() as es:
        ec = es.enter_context
        ec(nc.allow_low_precision(reason="bf16/fp8 pipeline; stats stay fp32"))
        cpool = ec(tc.tile_pool(name="const", bufs=1))
        embp = ec(tc.tile_pool(name="emb", bufs=4))
        xsp = ec(tc.tile_pool(name="xstate", bufs=2))
        actp = ec(tc.tile_pool(name="act1", bufs=1))
        lntp = ec(tc.tile_pool(name="lnt", bufs=2))
        rowp = ec(tc.tile_pool(name="rows", bufs=1))
        lnwp = ec(tc.tile_pool(name="lnw", bufs=2))
        x1p = ec(tc.tile_pool(name="x1p", bufs=1))
        big1 = ec(tc.tile_pool(name="big1", bufs=1))
        wqkp = ec(tc.tile_pool(name="wqk", bufs=1))
        wvp = ec(tc.tile_pool(name="wvp", bufs=1))
        wop = ec(tc.tile_pool(name="wop", bufs=1))
        wfp = ec(tc.tile_pool(name="wfp", bufs=1))
        smp = ec(tc.tile_pool(name="small", bufs=4))
        psp = ec(tc.tile_pool(name="psum", bufs=8, space="PSUM"))
        dramp = ec(tc.tile_pool(name="dram", bufs=2, space="DRAM"))
        if True:
            def ps_tile(name):
                return psp.tile([128, 512], F32, tag="ps", name=name)

            # ------------- constants (gather indices first: gathers gate all) --
            gids_sb = cpool.tile([128, 2 * NG], I32)
            nc.sync.dma_start(gids_sb[:], gids[:])
            posb_sb = actp.tile([128, 2 * H], F16, tag="xq", name="posb_sb")
            nc.sync.dma_start(posb_sb[:], posb[:])
            cwx_sb = cpool.tile([2 * NN, T], F32R)
            nc.sync.dma_start(cwx_sb[:], cwx[:])
            gc_sb = cpool.tile([2 * NN, H], F32R)
            nc.sync.dma_start(gc_sb[:], gcmat[:])
            cid_sb = cpool.tile([128, 132], F32R)
            nc.sync.dma_start(cid_sb[:], cid[:])
            ident = cid_sb[:, 0:128]
            ones_col = cid_sb[:, 128:129]
            hot_sb = cpool.tile([128, NH * NH], BF16)
            nc.sync.dma_start(hot_sb[:], hotmat[:])
            selm_sb = cpool.tile([NH, H], F32R)
            nc.sync.dma_start(selm_sb[:], selmat[:])
            ones_sb = cpool.tile([1, 256], F32R)
            nc.sync.dma_start(ones_sb[:], onesall[:])
            agoff_sb = cpool.tile([128, 1], I32)
            nc.sync.dma_start(agoff_sb[:], agoff[:])
            rm_t = cpool.tile([2, T], F32R)
            nc.sync.dma_start(rm_t[:], neg2[:])
            ln0w_sb = actp.tile([128, 2 * H], F32R, tag="xln", name="ln0w_sb")
            nc.sync.dma_start(ln0w_sb[:], ln0w[:])
            eps0 = cpool.tile([128, 1], F32)
            nc.vector.memset(eps0[:], float(H) * H * 1e-12)
            epsl = cpool.tile([128, 1], F32)
            nc.vector.memset(epsl[:], float(H) * H * 1e-5)
            nhalf = cpool.tile([128, 1], F32)
            nc.vector.memset(nhalf[:], -0.5)
            if FP8_FFN:
                rs1 = cpool.tile([128, 1], F32)
                nc.vector.memset(rs1[:], 1.0 / WS1)

            lnargs = dict(nc=nc, mybir=mybir, ps_tile=ps_tile, lnwp=lnwp,
                          lntp=lntp, rowp=rowp, gbw=gbw, ones_col=ones_col,
                          rm_t=rm_t, epsl=epsl, nhalf=nhalf)

            # ------------- embedding + cascade + LN0 (token-major) -------------
            x0tok = []
            for t in range(2):
                # multi-row gathers: buffer i holds GSPLIT[i] gathered rows/token
                gb = []
                o0 = 0
                for i, k in enumerate(GSPLIT):
                    # reuse e_t score-tile slots (disjoint lifetime, same size)
                    gt = big1.tile([128, k * H], F16, tag=f"e_{i}", name=f"g{t}_{i}")
                    for j in range(k):
                        # HW SWDGE only honors one offset per partition per op
                        nc.gpsimd.indirect_dma_start(
                            out=gt[:, j * H:(j + 1) * H],
                            out_offset=None,
                            in_=table[:],
                            in_offset=bass.IndirectOffsetOnAxis(
                                ap=gids_sb[:, t * NG + o0 + j:
                                           t * NG + o0 + j + 1],
                                axis=0),
                        )
                    gb.append(gt)
                    o0 += k
                # tree-reduce the 14 segments + posb
                red = []
                for i, k in enumerate(GSPLIT):
                    gt = gb[i]
                    nc.vector.tensor_tensor(gt[:, 0:H], gt[:, 0:H], gt[:, H:2 * H],
                                            op=OP.add)
                    if k == 4:
                        nc.vector.tensor_tensor(gt[:, 2 * H:3 * H], gt[:, 2 * H:3 * H],
                                                gt[:, 3 * H:4 * H], op=OP.add)
                    nc.vector.tensor_tensor(gt[:, 0:H], gt[:, 0:H], gt[:, 2 * H:3 * H],
                                            op=OP.add)
                    red.append(gt[:, 0:H])
                nc.vector.tensor_tensor(red[0], red[0], red[1], op=OP.add)
                nc.vector.tensor_tensor(red[2], red[2], red[3], op=OP.add)
                nc.vector.tensor_tensor(red[2], red[2],
                                        posb_sb[:, t * H:(t + 1) * H], op=OP.add)
                casc_ps = ps_tile(f"casc{t}")
                casc_ps2 = ps_tile(f"casc2_{t}")
                nc.tensor.matmul(casc_ps[:, 0:512],
                                 lhsT=cwx_sb[:, t * 128:(t + 1) * 128],
                                 rhs=gc_sb[:, 0:512], start=True, stop=True)
                nc.tensor.matmul(casc_ps2[:, 0:256],
                                 lhsT=cwx_sb[:, t * 128:(t + 1) * 128],
                                 rhs=gc_sb[:, 512:768], start=True, stop=True)
                xg = embp.tile([128, H], F32R, tag="emb", name=f"xg{t}")
                nc.vector.tensor_tensor(xg[:], red[0], red[2], op=OP.add)
                nc.vector.tensor_tensor(xg[:, 0:512], xg[:, 0:512], casc_ps[:, 0:512],
                                        op=OP.add)
                nc.vector.tensor_tensor(xg[:, 512:768], xg[:, 512:768],
                                        casc_ps2[:, 0:256], op=OP.add)
                # LN0 (token-major): var*H^2 = H*sum(x^2) - sum(x)^2
                s1 = smp.tile([128, 1], F32, tag="s1")
                nc.vector.tensor_reduce(s1[:], xg[:], axis=AX.X, op=OP.add)
                scr = embp.tile([128, H], F32R, tag="emb", name=f"scr{t}")
                s2 = smp.tile([128, 1], F32, tag="s2")
                nc.scalar.activation(scr[:], xg[:], AF.Square, accum_out=s2[:])
                s1sq = smp.tile([128, 1], F32, tag="s1sq")
                nc.scalar.activation(s1sq[:], s1[:], AF.Square)
                t1 = smp.tile([128, 1], F32, tag="t1")
                nc.vector.scalar_tensor_tensor(t1[:], s2[:], float(H), s1sq[:],
                                               op0=OP.mult, op1=OP.subtract)
                # rstd via exp(-0.5*ln(.)): keeps ACT in one table set (ln+exp)
                lnv = smp.tile([128, 1], F32, tag="lnv")
                nc.scalar.activation(lnv[:], t1[:], AF.Ln, bias=eps0[:, 0:1])
                rr0 = smp.tile([128, 1], F32, tag="rr0")
                nc.scalar.activation(rr0[:], lnv[:], AF.Exp, scale=nhalf[:, 0:1])
                mean = smp.tile([128, 1], F32, tag="mean")
                nc.vector.tensor_scalar_mul(mean[:], s1[:], 1.0 / H)
                nc.vector.tensor_scalar(xg[:], xg[:], mean[:, 0:1], rr0[:, 0:1],
                                        op0=OP.subtract, op1=OP.mult)
                # ln0w rows: [g*H | b]
                nc.vector.tensor_tensor(xg[:], xg[:], ln0w_sb[:, 0:H], op=OP.mult)
                xt = embp.tile([128, H], F32R, tag="emb", name=f"x0tok{t}")
                nc.vector.tensor_tensor(xt[:], xg[:], ln0w_sb[:, H:2 * H], op=OP.add)
                x0tok.append(xt)

            # bridge: transpose to feature-major contiguous x [128, HC*T]
            x0_all = xsp.tile([128, HC, T], F32R, tag="x_all", name="x0_all")
            for c in range(HC):
                for t in range(2):
                    tp = ps_tile(f"br{c}_{t}")
                    nc.tensor.matmul(tp[:, 0:128],
                                     lhsT=x0tok[t][:, c * 128:(c + 1) * 128],
                                     rhs=ident[:], start=True, stop=True)
                    nc.vector.tensor_copy(x0_all[:, c, t * 128:(t + 1) * 128],
                                          tp[:, 0:128])
            xcur_all = x0_all
            xcur = [xcur_all[:, c, :] for c in range(HC)]

            # ------------- transformer layers -------------
            for l in [ll % NL for ll in range(NL * reps)]:
                # ---- weight slab DMAs (few, large; pools double-buffer) ----
                wq_sb = wqkp.tile([128, 2, HC, 384], BF16, tag="wq", name="wq_sb")
                nc.sync.dma_start(wq_sb[:], wq_m[l])
                wk_sb = wqkp.tile([128, 2, HC, 384], BF16, tag="wk", name="wk_sb")
                nc.sync.dma_start(wk_sb[:], wk_m[l])
                wv_sb = wvp.tile([128, 2 * HC, 384], BF16, tag="wv", name="wv_sb")
                nc.sync.dma_start(wv_sb[:], wv_m[l])
                wo_sb = wop.tile([128, 2, HC, 384], BF16, tag="wo", name="wo_sb")
                nc.sync.dma_start(wo_sb[:], wo_m[l])
                if FP8_FFN:
                    w1_sb = wfp.tile([128, FC, 3, 2, 128], FP8, tag="w1",
                                     name="w1_sb")
                    nc.sync.dma_start(w1_sb[:], w1f[l])
                    w2_sb = wfp.tile([128, FC // 2, HC, 2, 128], FP8, tag="w2",
                                     name="w2_sb")
                    nc.sync.dma_start(w2_sb[:], w2f[l])
                    f2b_sb = lnwp.tile([1, H], F32R, tag="f2b", name="f2b_sb")
                    nc.scalar.dma_start(f2b_sb[:], f2br[l])

                # ---- AllGather x within pairs (contiguous, single ops) ----
                ag_in = dramp.tile([128, HC * T], BF16, tag="ag_in")
                ag_out = dramp.tile([256, HC * T], BF16, tag="ag_out")
                xq_all = actp.tile([128, HC, T], BF16, tag="xq", name="xq_all")
                xq = [xq_all[:, c, :] for c in range(HC)]
                # GpSimd is idle during layers; casts off the DVE hot path.
                # Two halves so the first Q/K matmuls and the collective input
                # DMA start before the second half has cast.
                for hb in range(2):
                    nc.gpsimd.tensor_copy(xq_all[:, hb * 3:hb * 3 + 3, :],
                                          xcur_all[:, hb * 3:hb * 3 + 3, :])
                    nc.scalar.dma_start(ag_in[:, hb * 3 * T:(hb + 1) * 3 * T],
                                        xq_all[:, hb * 3:hb * 3 + 3, :])
                import os as _os
                if sim or _os.environ.get('KERNEL_NOCOLL'):
                    nc.sync.dma_start(ag_out[0:128, :], ag_in[:])
                    nc.sync.dma_start(ag_out[128:256, :], ag_in[:])
                else:
                    nc.gpsimd.collective_compute(
                        "AllGather", OP.bypass,
                        replica_groups=[[0, 1], [2, 3], [4, 5], [6, 7]],
                        ins=[ag_in[:].opt()], outs=[ag_out[:].opt()],
                    )

                bv_sb = smp.tile([128, 36], F32, tag="bv")
                nc.scalar.dma_start(bv_sb[:], bvec[l])

                # ---- OWN-half phase (no collective dependency) ----
                # Q (own tokens) and K own-half columns
                q_t, k_t = [], []
                for c in range(HC):
                    qt_ = actp.tile([128, T], BF16, tag=f"q_{c}", name=f"q_{c}")
                    q_t.append(qt_)
                    kt_ = big1.tile([128, TK], BF16, tag=f"k_{c}", name=f"k_{c}")
                    k_t.append(kt_)
                for ms in range(2):
                    for mo in range(3):
                        m = ms * 3 + mo
                        qp = ps_tile(f"qp{m}")
                        for k in range(HC):
                            nc.tensor.matmul(qp[:, 0:T],
                                             lhsT=wq_sb[:, ms, k,
                                                        mo * 128:(mo + 1) * 128],
                                             rhs=xq[k][:],
                                             start=(k == 0), stop=(k == HC - 1))
                        nc.scalar.copy(q_t[m][:], qp[:, 0:T])
                        kp = ps_tile(f"kp{m}")
                        for k in range(HC):
                            nc.tensor.matmul(kp[:, 0:T],
                                             lhsT=wk_sb[:, ms, k,
                                                        mo * 128:(mo + 1) * 128],
                                             rhs=xq[k][:],
                                             start=(k == 0), stop=(k == HC - 1))
                        nc.scalar.copy(k_t[m][:, 0:T], kp[:, 0:T])

                # V own tiles (kv tiles 0,1), token-major [kv, d] bf16
                v_tm = []
                for kt in range(KVT):
                    vt_ = actp.tile([128, H], BF16, tag=f"v_{kt}", name=f"v_{kt}")
                    v_tm.append(vt_)
                for half in range(2):
                    vps = [ps_tile(f"vp{half}_{kt}") for kt in range(2)]
                    for k in range(HC):
                        for kt in range(2):
                            nc.tensor.matmul(
                                vps[kt][:, 0:384],
                                lhsT=xq[k][:, kt * 128:(kt + 1) * 128],
                                rhs=wv_sb[:, half * HC + k, :],
                                start=(k == 0), stop=(k == HC - 1))
                    for kt in range(2):
                        nc.vector.tensor_copy(
                            v_tm[kt][:, half * 384:(half + 1) * 384],
                            vps[kt][:, 0:384])

                # scores + exp (head pairs share a psum bank; one exp per pair)
                # sums accumulate into 32-aligned psum rows: su bank A rows
                # {0,32,64,96} for m=0..3, bank B rows {0,32} for m=4,5
                e_t = []
                for kt in range(KVT):
                    et_ = big1.tile([128, NH * T], BF16, tag=f"e_{kt}", name=f"e_{kt}")
                    e_t.append(et_)
                su_ps = ps_tile("sums")
                nmm = [0]

                def pair_scores(m, kt):
                    # per-head psum banks: matmul outputs must start at a
                    # bank base (column-offset outputs hang the device)
                    for half in range(2):
                        hh = 2 * m + half
                        sp = ps_tile(f"sc{hh}_{kt}")
                        nc.tensor.matmul(
                            sp[:, 0:T],
                            lhsT=k_t[m][half * 64:half * 64 + 64,
                                        kt * 128:(kt + 1) * 128],
                            rhs=q_t[m][half * 64:half * 64 + 64, :],
                            start=True, stop=True)
                        nc.scalar.activation(
                            e_t[kt][:, hh * T:(hh + 1) * T], sp[:, 0:T], AF.Exp)
                    for half in range(2):
                        hh = 2 * m + half
                        nc.tensor.matmul(su_ps[0:NH, 0:T],
                                         lhsT=hot_sb[:, hh * NH:(hh + 1) * NH],
                                         rhs=e_t[kt][:, hh * T:(hh + 1) * T],
                                         start=(nmm[0] == 0), stop=(nmm[0] == 47))
                        nmm[0] += 1

                for m in range(HC):
                    for kt in range(2):
                        pair_scores(m, kt)

                # ---- FOREIGN-half phase: xf = (ag0 + ag1) - xq (exact) ----
                xf_all = actp.tile([128, HC, T], BF16, tag="xf", name="xf_all")
                xf = [xf_all[:, c, :] for c in range(HC)]
                # two pipelined halves: foreign-K matmuls start on the first
                # half while the second is still in flight
                for hb in range(2):
                    sl = slice(hb * 3 * T, (hb + 1) * 3 * T)
                    h0 = lntp.tile([128, 3 * T], BF16, tag=f"agh0_{hb}", bufs=1)
                    h1 = lntp.tile([128, 3 * T], BF16, tag=f"agh1_{hb}", bufs=1)
                    nc.scalar.dma_start(h0[:], ag_out[0:128, sl])
                    nc.sync.dma_start(h1[:], ag_out[128:256, sl])
                    nc.vector.tensor_tensor(xf_all[:, hb * 3:hb * 3 + 3, :],
                                            h0[:], h1[:], op=OP.add)
                    nc.vector.tensor_tensor(xf_all[:, hb * 3:hb * 3 + 3, :],
                                            xf_all[:, hb * 3:hb * 3 + 3, :],
                                            xq_all[:, hb * 3:hb * 3 + 3, :],
                                            op=OP.subtract)

                # K foreign-half columns
                for ms in range(2):
                    for mo in range(3):
                        m = ms * 3 + mo
                        kp = ps_tile(f"kf{m}")
                        for k in range(HC):
                            nc.tensor.matmul(kp[:, 0:T],
                                             lhsT=wk_sb[:, ms, k,
                                                        mo * 128:(mo + 1) * 128],
                                             rhs=xf[k][:],
                                             start=(k == 0), stop=(k == HC - 1))
                        nc.scalar.copy(k_t[m][:, T:TK], kp[:, 0:T])

                # V foreign tiles (kv tiles 2,3)
                for half in range(2):
                    vps = [ps_tile(f"vpf{half}_{kt}") for kt in range(2)]
                    for k in range(HC):
                        for kt in range(2):
                            nc.tensor.matmul(
                                vps[kt][:, 0:384],
                                lhsT=xf[k][:, kt * 128:(kt + 1) * 128],
                                rhs=wv_sb[:, half * HC + k, :],
                                start=(k == 0), stop=(k == HC - 1))
                    for kt in range(2):
                        nc.vector.tensor_copy(
                            v_tm[2 + kt][:, half * 384:(half + 1) * 384],
                            vps[kt][:, 0:384])

                # scores + exp + sums for foreign kv tiles
                for m in range(HC):
                    for kt in range(2, KVT):
                        pair_scores(m, kt)

                rec12 = rowp.tile([NH, T], F32R, tag="rec12")
                nc.vector.reciprocal(rec12[:], su_ps[0:NH, 0:T])

                # ---- attn = V^T @ E, normalized ----
                attn = []
                for m in range(HC):
                    ap_ = ps_tile(f"att{m}")
                    for half in range(2):
                        for kt in range(KVT):
                            hh = 2 * m + half
                            nc.tensor.matmul(
                                ap_[half * 64:half * 64 + 64, 0:T],
                                lhsT=v_tm[kt][:, hh * 64:(hh + 1) * 64],
                                rhs=e_t[kt][:, hh * T:(hh + 1) * T],
                                start=(kt == 0), stop=(kt == KVT - 1))
                    rb_ps = ps_tile(f"rb{m}")
                    nc.tensor.matmul(rb_ps[:, 0:T],
                                     lhsT=selm_sb[:, m * 128:(m + 1) * 128],
                                     rhs=rec12[:], start=True, stop=True)
                    rb = lntp.tile([128, T], F32, tag="rb")
                    nc.scalar.copy(rb[:], rb_ps[:, 0:T])
                    at = actp.tile([128, T], BF16, tag=f"attn_{m}", name=f"attn_{m}")
                    nc.vector.tensor_tensor(at[:], ap_[:, 0:T], rb[:], op=OP.mult)
                    attn.append(at)

                # ---- out-proj + bias + residual ----
                x1_all = x1p.tile([128, HC, T], F32R, tag="xt_all", name="x1_all")
                x1 = [x1_all[:, m, :] for m in range(HC)]
                for ms in range(2):
                    for mo in range(3):
                        m = ms * 3 + mo
                        op_ = ps_tile(f"op{m}")
                        for k in range(HC):
                            nc.tensor.matmul(op_[:, 0:T],
                                             lhsT=wo_sb[:, ms, k,
                                                        mo * 128:(mo + 1) * 128],
                                             rhs=attn[k][:],
                                             start=(k == 0), stop=(k == HC - 1))
                        nc.vector.scalar_tensor_tensor(
                            x1[m][:], op_[:, 0:T], bv_sb[:, m:m + 1], xcur[m][:],
                            op0=OP.add, op1=OP.add)

                # ---- LN1 ----
                xln_all, xln = _layer_norm(xin=x1, lni=l * 2, outpool=actp,
                                           outtag="xln", **lnargs)

                # ---- FFN ----
                if FP8_FFN:
                    # cast LN1 out to fp8 (contiguous [128, HC, T])
                    xq8 = actp.tile([128, HC, T], FP8, tag="xq8", name="xq8")
                    nc.gpsimd.tensor_copy(xq8[:], xln_all[:])
                    f2ps = [ps_tile(f"f2ps_{m}") for m in range(HC)]
                    f2ps = [t[:, 0:T] for t in f2ps]
                    # ff2 bias row (scaled by WS2 host-side), rank-1 matmul
                    for m in range(HC):
                        nc.tensor.matmul(f2ps[m][:],
                                         lhsT=f2b_sb[0:1, m * 128:(m + 1) * 128],
                                         rhs=ones_sb[0:1, 0:T],
                                         start=True, stop=False,
                                         skip_group_check=True)
                    fm_q = {}
                    for kp2 in range(FC // 2):
                        fmp = lntp.tile([128, 2, T], FP8, tag="ffm",
                                        name=f"ffm_{kp2}")
                        for j in range(2):
                            fo = 2 * kp2 + j
                            fp_ = ps_tile(f"fp{fo}")
                            for kp in range(3):
                                nc.tensor.matmul(
                                    fp_[:, 0:T],
                                    lhsT=w1_sb[:, fo, kp, :, :],
                                    rhs=xq8[:, 2 * kp:2 * kp + 2, :],
                                    start=(kp == 0), stop=(kp == 2),
                                    perf_mode=DR)
                            nc.scalar.activation(fmp[:, j, :], fp_[:, 0:T],
                                                 AF.Relu,
                                                 bias=bv_sb[:, 6 + fo:7 + fo],
                                                 scale=rs1[:, 0:1])
                        fm_q[kp2] = fmp
                        for m in range(HC):
                            nc.tensor.matmul(
                                f2ps[m][:],
                                lhsT=w2_sb[:, kp2, m, :, :],
                                rhs=fmp[:, 0:2, :],
                                start=False, stop=(kp2 == FC // 2 - 1),
                                perf_mode=DR, skip_group_check=True)
                    x2_all = xsp.tile([128, HC, T], F32R, tag="x_all",
                                      name="x2_all")
                    x2 = [x2_all[:, m, :] for m in range(HC)]
                    for m in range(HC):
                        nc.vector.scalar_tensor_tensor(
                            x2[m][:], f2ps[m][:], 1.0 / WS2, xln[m][:],
                            op0=OP.mult, op1=OP.add)
                else:
                    xlnb = []
                    for c in range(HC):
                        xb_ = actp.tile([128, T], BF16, tag=f"xlnb_{c}",
                                        name=f"xlnb_{c}")
                        nc.gpsimd.tensor_copy(xb_[:], xln[c][:])
                        xlnb.append(xb_)
                    f2ps = [ps_tile(f"f2ps_{m}") for m in range(HC)]
                    f2ps = [t[:, 0:T] for t in f2ps]
                    # ff1 runs one mid-chunk ahead of ff2 so relu never stalls PE
                    fm_q = {}
                    wsl_q = {}

                    def emit_f2(fo):
                        for m in range(HC):
                            nc.tensor.matmul(f2ps[m][:],
                                             lhsT=wsl_q[fo][:, m * 128:(m + 1) * 128],
                                             rhs=fm_q[fo][:],
                                             start=(fo == 0), stop=(fo == FC - 1))
                    for sl in range(8):
                        fsl = wfp.tile([128, HC, 384], BF16, tag="w1slab", bufs=4)
                        nc.sync.dma_start(fsl[:], w1_s[l, sl])
                        for mo in range(3):
                            fo = sl * 3 + mo
                            fp = ps_tile(f"fp{fo}")
                            for k in range(HC):
                                nc.tensor.matmul(fp[:, 0:T],
                                                 lhsT=fsl[:, k,
                                                          mo * 128:(mo + 1) * 128],
                                                 rhs=xlnb[k][:],
                                                 start=(k == 0), stop=(k == HC - 1))
                            fm = lntp.tile([128, T], BF16, tag="ffm",
                                           name=f"ffm_{fo}")
                            nc.scalar.activation(fm[:], fp[:, 0:T], AF.Relu,
                                                 bias=bv_sb[:, 6 + fo:7 + fo])
                            fm_q[fo] = fm
                            wsl = wfp.tile([128, H], BF16, tag="w2slab", bufs=8)
                            nc.sync.dma_start(wsl[:], w2_s[l, fo])
                            wsl_q[fo] = wsl
                            if fo >= 1:
                                emit_f2(fo - 1)
                    emit_f2(FC - 1)
                    x2_all = xsp.tile([128, HC, T], F32R, tag="x_all",
                                      name="x2_all")
                    x2 = [x2_all[:, m, :] for m in range(HC)]
                    for m in range(HC):
                        nc.vector.scalar_tensor_tensor(
                            x2[m][:], f2ps[m][:], bv_sb[:, 30 + m:31 + m],
                            xln[m][:], op0=OP.add, op1=OP.add)

                # ---- LN2 -> next x ----
                xcur_all, xcur = _layer_norm(xin=x2, lni=l * 2 + 1, outpool=xsp,
                                             outtag="x_all", **lnargs)

            # ------------- output -------------
            for c in range(HC):
                nc.scalar.dma_start(y_out[c * 128:(c + 1) * 128, :], xcur[c][:])

    nc.compile()
    return nc


def _layer_norm(nc, mybir, ps_tile, lnwp, lntp, rowp, gbw, ones_col, rm_t, epsl,
                nhalf, xin, lni, outpool, outtag):
    """Feature-major layernorm over 6 chunks [128, T], writing a contiguous
    [128, 6, T] output tile (views returned).
    gbw rows: [g*H, g, b]; rr0 = 1/(H*std); a = gH (x) rr0; b_ps = g (x) rr0*S - b.
    """
    F32 = mybir.dt.float32
    F32R = mybir.dt.float32r
    AF = mybir.ActivationFunctionType
    OP = mybir.AluOpType
    H = 768
    gb = lnwp.tile([2, H], F32R, tag="gb", name=f"gb{lni}")
    nc.scalar.dma_start(gb[:], gbw[lni, 0:2])
    gh = lnwp.tile([1, H], F32R, tag="gh", name=f"gh{lni}")
    nc.scalar.dma_start(gh[:], gbw[lni, 2:3])
    s_ps = ps_tile(f"lns{lni}")
    q_ps = ps_tile(f"lnq{lni}")
    for c in range(6):
        sq = lntp.tile([128, 256], F32R, tag="lnsq")
        nc.scalar.activation(sq[:], xin[c][:], AF.Square)
        nc.tensor.matmul(s_ps[0:1, 0:256], lhsT=ones_col[:], rhs=xin[c][:],
                         start=(c == 0), stop=(c == 5))
        nc.tensor.matmul(q_ps[0:1, 0:256], lhsT=ones_col[:], rhs=sq[:],
                         start=(c == 0), stop=(c == 5))
    s2 = rowp.tile([1, 256], F32, tag="ls2")
    nc.scalar.activation(s2[:], s_ps[0:1, 0:256], AF.Square)
    t1 = rowp.tile([1, 256], F32, tag="lt1")
    nc.vector.scalar_tensor_tensor(t1[:], q_ps[0:1, 0:256], float(H), s2[:],
                                   op0=OP.mult, op1=OP.subtract)
    lnv = rowp.tile([1, 256], F32, tag="llnv")
    nc.scalar.activation(lnv[:], t1[:], AF.Ln, bias=epsl[0:1, 0:1])
    rr = rowp.tile([1, 256], F32R, tag="lr")
    nc.scalar.activation(rr[:], lnv[:], AF.Exp, scale=nhalf[0:1, 0:1])
    nc.vector.tensor_tensor(rm_t[0:1, :], rr[:], s_ps[0:1, 0:256], op=OP.mult)
    out_all = outpool.tile([128, 6, 256], F32R, tag=outtag,
                           name=f"{outtag}{lni}")
    out = []
    for c in range(6):
        a_ps = ps_tile(f"lna{lni}_{c}")
        nc.tensor.matmul(a_ps[:, 0:256], lhsT=gh[0:1, c * 128:(c + 1) * 128],
                         rhs=rr[:], start=True, stop=True)
        b_ps = ps_tile(f"lnb{lni}_{c}")
        nc.tensor.matmul(b_ps[:, 0:256], lhsT=gb[:, c * 128:(c + 1) * 128],
                         rhs=rm_t[:], start=True, stop=True)
        tt = lntp.tile([128, 256], F32R, tag="lnt")
        nc.vector.tensor_tensor(tt[:], xin[c][:], a_ps[:, 0:256], op=OP.mult)
        nc.vector.tensor_tensor(out_all[:, c, :], tt[:], b_ps[:, 0:256],
                                op=OP.subtract)
        out.append(out_all[:, c, :])
    return out_all, out


def _host_pack(inputs):
    import ml_dtypes
    f32 = np.float32
    f16 = np.float16
    bf = ml_dtypes.bfloat16
    f8 = ml_dtypes.float8_e4m3
    tok = np.asarray(inputs['tok_emb'], f32)
    pos = np.asarray(inputs['pos_emb'], f32)
    node = np.asarray(inputs['node_emb'], f32)
    cw_W = np.asarray(inputs['cw_W'], f32)
    cw_b = np.asarray(inputs['cw_b'], f32)
    fus_W = np.asarray(inputs['fus_W'], f32)
    fus_b = np.asarray(inputs['fus_b'], f32)
    ln_g = np.asarray(inputs['ln_g'], f32)
    ln_b = np.asarray(inputs['ln_b'], f32)
    iW = np.asarray(inputs['attn_in_W'], f32)
    ib = np.asarray(inputs['attn_in_b'], f32)
    oW = np.asarray(inputs['attn_out_W'], f32)
    ob = np.asarray(inputs['attn_out_b'], f32)
    f1W = np.asarray(inputs['ff1_W'], f32)
    f1b = np.asarray(inputs['ff1_b'], f32)
    f2W = np.asarray(inputs['ff2_W'], f32)
    f2b = np.asarray(inputs['ff2_b'], f32)
    g1 = np.asarray(inputs['ln1_g'], f32)
    b1 = np.asarray(inputs['ln1_b'], f32)
    g2 = np.asarray(inputs['ln2_g'], f32)
    b2 = np.asarray(inputs['ln2_b'], f32)
    input_ids = np.asarray(inputs['input_ids']).astype(np.int64)
    ccids = np.asarray(inputs['cascade_concept_ids']).astype(np.int64)
    cwts = np.asarray(inputs['cascade_weights'], f32)
    cmask = np.asarray(inputs['cascade_mask']).astype(bool)

    fw3 = fus_W.reshape(H, NN, EE)
    table = np.empty((TROWS, H), f16)
    table[:V] = tok.astype(f16)
    tn = np.matmul(node[None, :, :], fw3.transpose(1, 2, 0))
    table[V:V + NN * NCC] = tn.reshape(NN * NCC, H).astype(f16)
    table[ZROW] = 0.0
    G = np.einsum('e,hne->nh', cw_W[:, 0], fw3)
    C = np.einsum('e,hne->nh', cw_b, fw3)
    gcmat = np.concatenate([G, C], axis=0).astype(f32)

    cid = np.zeros((128, 132), f32)
    cid[:, :128] = np.eye(128, dtype=f32)
    cid[:, 128] = 1.0
    hotm = np.zeros((128, NH * NH), bf)
    for hh_ in range(NH):
        hotm[:, hh_ * NH + hh_] = 1.0
    selm = np.zeros((NH, H), f32)
    for m_ in range(HC):
        selm[2 * m_, m_ * 128:m_ * 128 + 64] = 1.0
        selm[2 * m_ + 1, m_ * 128 + 64:(m_ + 1) * 128] = 1.0
    onesall = np.ones((1, 256), f32)
    neg2 = np.full((2, T), -1.0, f32)
    ln0w = np.empty((128, 2 * H), f32)
    ln0w[:, :H] = np.broadcast_to(ln_g[None, :] * H, (128, H))
    ln0w[:, H:] = np.broadcast_to(ln_b[None, :], (128, H))
    gbw = np.empty((NL * 2, 3, H), f32)
    for l in range(NL):
        gbw[2 * l, 0], gbw[2 * l, 1], gbw[2 * l, 2] = g1[l], b1[l], g1[l] * H
        gbw[2 * l + 1, 0], gbw[2 * l + 1, 1], gbw[2 * l + 1, 2] = \
            g2[l], b2[l], g2[l] * H

    def mslab(wt, nslab):
        K, M = wt.shape
        w = M // nslab
        a = wt.reshape(K // 128, 128, M).transpose(1, 0, 2)
        return np.stack([a[:, :, i * w:(i + 1) * w] for i in range(nslab)], 0)

    wq_m = np.empty((NL, 128, 2, HC, 384), bf)
    wk_m = np.empty((NL, 128, 2, HC, 384), bf)
    wo_m = np.empty((NL, 128, 2, HC, 384), bf)
    wv_m = np.empty((NL, 128, 2 * HC, 384), bf)
    bvec = np.empty((NL, 128, 36), f32)
    if FP8_FFN:
        w1f = np.empty((NL, 128, FC, 3, 2, 128), f8)
        w2f = np.empty((NL, 128, FC // 2, HC, 2, 128), f8)
        f2br = np.empty((NL, 1, H), f32)
    else:
        w1_s = np.empty((NL, 8, 128, HC, 384), bf)
        w2_s = np.empty((NL, FC, 128, H), bf)
    for l in range(NL):
        wq_t = iW[l, 0:H, :].T * (1.0 / np.sqrt(HD))
        wk_t = iW[l, H:2 * H, :].T
        wv_t = iW[l, 2 * H:3 * H, :].T
        wq_m[l] = mslab(wq_t, 2).transpose(1, 0, 2, 3)
        wk_m[l] = mslab(wk_t, 2).transpose(1, 0, 2, 3)
        wo_m[l] = mslab(oW[l].T, 2).transpose(1, 0, 2, 3)
        for half in range(2):
            wv_m[l, :, half * HC:(half + 1) * HC] = \
                wv_t[:, half * 384:(half + 1) * 384].reshape(
                    HC, 128, 384).transpose(1, 0, 2)
        if FP8_FFN:
            # w1f[p, fo, kp, j, o] = f1W[fo*128+o, (2kp+j)*128+p] * WS1
            a = (f1W[l] * WS1).reshape(FC, 128, 6, 128)   # [fo, o, s, p]
            a = a.reshape(FC, 128, 3, 2, 128)             # [fo, o, kp, j, p]
            w1f[l] = a.transpose(4, 0, 2, 3, 1).astype(f8)
            # w2f[p, kp, m, j, o] = f2W[m*128+o, (2kp+j)*128+p] * WS2
            b = (f2W[l] * WS2).reshape(HC, 128, FC // 2, 2, 128)
            w2f[l] = b.transpose(4, 2, 0, 3, 1).astype(f8)
            f2br[l, 0] = f2b[l] * WS2
        else:
            w1_s[l] = mslab(f1W[l].T, 8)
            w2_s[l] = f2W[l].T.reshape(FC, 128, H).astype(bf)
        ob2 = ob[l] + oW[l] @ ib[l, 2 * H:3 * H]
        bvec[l, :, 0:6] = ob2.reshape(HC, 128).T
        bvec[l, :, 6:30] = f1b[l].reshape(FC, 128).T
        bvec[l, :, 30:36] = f2b[l].reshape(HC, 128).T

    shared = dict(table=table, gcmat=gcmat, cid=cid, hotmat=hotm,
                  selmat=selm, onesall=onesall, neg2=neg2, ln0w=ln0w, gbw=gbw,
                  wq_m=wq_m, wk_m=wk_m, wo_m=wo_m, wv_m=wv_m, bvec=bvec)
    if FP8_FFN:
        shared.update(w1f=w1f, w2f=w2f, f2br=f2br)
    else:
        shared.update(w1_s=w1_s, w2_s=w2_s)

    cwm = (cwts * cmask).astype(f32)
    in_maps = []
    for r in range(NCORES):
        b, hh = r // 2, r % 2
        ssl = slice(hh * T, (hh + 1) * T)
        sidx = np.arange(S)[ssl]
        gid = np.empty((128, 2 * NG), np.int32)
        pb = np.empty((128, 2 * H), f16)
        for t in range(2):
            rows = sidx[t * 128:(t + 1) * 128]
            gid[:, t * NG + 0] = input_ids[b, rows]
            for n in range(NN):
                cc = V + n * NCC + ccids[rows, n]
                cc = np.where(cmask[rows, n], cc, ZROW)
                gid[:, t * NG + 1 + n] = cc
            pb[:, t * H:(t + 1) * H] = (pos[rows] + fus_b[None, :]).astype(f16)
        cwxv = np.concatenate([cwm[ssl].T, cmask[ssl].T.astype(f32)], 0)
        m = dict(shared)
        m['gids'] = gid
        m['posb'] = pb
        m['cwx'] = np.ascontiguousarray(cwxv)
        # foreign-half row offsets into ag_out: partner block +(own partition)
        m['agoff'] = ((1 - hh) * 128 + np.arange(128, dtype=np.int32)
                      ).reshape(128, 1)
        in_maps.append(m)
    return in_maps


def _make_runner(reps=1):
    """Build nc once and return fn(in_maps) -> list of per-core result dicts,
    with the jitted executable cached for repeat timing."""
    import jax
    from jax.sharding import Mesh, PartitionSpec
    from jax.experimental.shard_map import shard_map
    import concourse.mybir as mybir
    from concourse import bass2jax
    from concourse.bass2jax import _bass_exec_p, install_neuronx_cc_hook, \
        partition_id_tensor

    nc = _build_nc(reps)
    install_neuronx_cc_hook()
    partition_name = nc.partition_id_tensor.name if nc.partition_id_tensor else None
    in_names, out_names, out_avals, zero_outs = [], [], [], []
    for alloc in nc.m.functions[0].allocations:
        if not isinstance(alloc, mybir.MemoryLocationSet):
            continue
        name = alloc.memorylocations[0].name
        if alloc.kind == "ExternalInput":
            if name != partition_name:
                in_names.append(name)
        elif alloc.kind == "ExternalOutput":
            out_names.append(name)
            shape = tuple(alloc.tensor_shape)
            dtype = mybir.dt.np(alloc.dtype)
            out_avals.append(jax.core.ShapedArray(shape, dtype))
            zero_outs.append(np.zeros(shape, dtype))
    n_params = len(in_names)
    n_outs = len(out_avals)
    all_names = in_names + out_names + ([partition_name] if partition_name else [])
    donate = tuple(range(n_params, n_params + n_outs))

    def _body(*args):
        operands = list(args)
        if partition_name is not None:
            operands.append(partition_id_tensor())
        outs = _bass_exec_p.bind(
            *operands,
            out_avals=tuple(out_avals),
            in_names=tuple(all_names),
            out_names=tuple(out_names),
            lowering_input_output_aliases=(),
            sim_require_finite=True,
            sim_require_nnan=True,
            nc=nc,
        )
        return tuple(outs)

    devices = jax.devices()[:NCORES]
    mesh = Mesh(np.asarray(devices), ("core",))
    in_specs = (PartitionSpec("core"),) * (n_params + n_outs)
    out_specs = (PartitionSpec("core"),) * len(out_names)
    sharded = jax.jit(
        shard_map(_body, mesh=mesh, in_specs=in_specs, out_specs=out_specs,
                  check_rep=False),
        donate_argnums=donate, keep_unused=True)

    def runner(in_maps, n_iters=1, dev_inputs=None):
        import time as _time
        if dev_inputs is None:
            concat_in = [np.concatenate([np.asarray(in_maps[c][nm])
                                         for c in range(NCORES)], axis=0)
                         for nm in in_names]
            dev_inputs = [jax.device_put(a) for a in concat_in]
        times = []
        out_arrs = None
        for _ in range(n_iters):
            concat_zeros = [np.zeros((NCORES * z.shape[0], *z.shape[1:]), z.dtype)
                            for z in zero_outs]
            t0 = _time.time()
            out_arrs = sharded(*dev_inputs, *concat_zeros)
            jax.block_until_ready(out_arrs)
            times.append(_time.time() - t0)
        results = [
            {nm: np.asarray(out_arrs[i]).reshape(NCORES, *out_avals[i].shape)[c]
             for i, nm in enumerate(out_names)}
            for c in range(NCORES)
        ]
        return results, times, dev_inputs

    return runner


def _run(inputs, trace=False, n_iters=1, reps=1):
    key = f'runner{reps}'
    if key not in _CACHE:
        _CACHE[key] = _make_runner(reps)
    runner = _CACHE[key]
    in_maps = _host_pack(inputs)
    results, times, _ = runner(in_maps, n_iters=n_iters)
    out = np.empty((B, S, H), np.float32)
    for r in range(NCORES):
        b, hh = r // 2, r % 2
        y = np.asarray(results[r]['y'])
        out[b, hh * T:(hh + 1) * T, :] = y.T

    class Res:
        pass
    res = Res()
    res.times = times
    res.exec_time_ns = None
    return out, res


def kernel(**inputs):
    out, _ = _run(inputs)
    return out
